# revision 3
# baseline (speedup 1.0000x reference)
"""Trainium2 Bass kernel for nn_Diagnosis (gnn_message_passing).

Model (per edge e, concepts k=0..3):
  stu_part  = (stu_fusion  @ Ws_a.T)[stu_idx]           [E, 128]
  item_part = (item_fusion @ Wi_a.T)[item_idx]          [E, 128]
  conc_s    = (concept_fusion @ Ws_b.T)[conc_idx]       [E, 4, 128]
  conc_i    = (concept_fusion @ Wi_b.T)[conc_idx]       [E, 4, 128]
  pred[e,k] = sigmoid(w . (sig(stu_part+conc_s) - sig(item_part+conc_i)) + b)
  out[e]    = mean_k pred[e,k]

Device strategy (data-parallel over edges, 8 cores, all-bf16 on chip):
  - channels-on-partitions layout [128 c, pairs] via transpose-mode dma_gather
  - raw stu/item rows gathered from bf16-padded tables, projected per-tile on PE
  - concept tables projected once on device into DRAM, then row-gathered per pair
  - sig(stu)-sig(item) = sig(stu) + sig(-item) - 1: item side uses ACT scale=-1,
    the constant folds into the final bias  b_eff = b - sum(w)
  - dot with w via PE: lhsT = sigmoid tile [128c x 128pairs], rhs = w [128,1],
    both sides accumulated into one PSUM column -> final sigmoid reads PSUM
  - k-mean on DVE (strided reduce), output [128, 256] f32 per core
"""
import numpy as np
import ml_dtypes
from contextlib import ExitStack

import concourse.bacc as bacc
import concourse.tile as tile
import concourse.mybir as mybir
from concourse.bass_utils import run_bass_kernel_spmd
from concourse import library_config

bf16 = ml_dtypes.bfloat16

# ---- problem constants (hardcoded per contest rules) ----
N_STU, N_ITEM, N_CONC = 10000, 50000, 2048
EDGES, K, EMB, CNUM = 250000, 4, 64, 128
NCORES = 8
E_CORE = EDGES // NCORES          # 31250
ITEM_SPLIT = 32768                # int16 index range split for the item table
N_ITEM_HI = N_ITEM - ITEM_SPLIT   # 17232
TILE_E = 1024                     # edges per super-tile
T_LO, T_HI = 21, 11               # super-tiles for item<32768 / >=32768 groups
T_TOT = T_LO + T_HI               # 32
LO_CAP, HI_CAP = T_LO * TILE_E, T_HI * TILE_E   # 21504, 11264
E_PAD = T_TOT * TILE_E            # 32768 padded edges per core
NPAIR = 4 * TILE_E                # 4096 pairs per super-tile
NVG = N_CONC // 512               # conc projection groups of 512 rows

_CACHE = {}


def _build_nc():
    nc = bacc.Bacc("TRN2", target_bir_lowering=False, debug=False)
    dt = mybir.dt

    d_stu = nc.dram_tensor("stu_pad", [N_STU, 128], dt.bfloat16, kind="ExternalInput")
    d_ilo = nc.dram_tensor("item_lo", [ITEM_SPLIT, 128], dt.bfloat16, kind="ExternalInput")
    d_ihi = nc.dram_tensor("item_hi", [N_ITEM_HI, 128], dt.bfloat16, kind="ExternalInput")
    d_cT = nc.dram_tensor("concT", [EMB, N_CONC], dt.bfloat16, kind="ExternalInput")
    d_wsa = nc.dram_tensor("WsaT", [EMB, 128], dt.bfloat16, kind="ExternalInput")
    d_wsb = nc.dram_tensor("WsbT", [EMB, 128], dt.bfloat16, kind="ExternalInput")
    d_wia = nc.dram_tensor("WiaT", [EMB, 128], dt.bfloat16, kind="ExternalInput")
    d_wib = nc.dram_tensor("WibT", [EMB, 128], dt.bfloat16, kind="ExternalInput")
    d_w = nc.dram_tensor("w", [128, 1], dt.bfloat16, kind="ExternalInput")
    d_be = nc.dram_tensor("beff", [128, 1], dt.float32, kind="ExternalInput")
    d_sidx = nc.dram_tensor("sidx", [128, T_TOT * 64], dt.int16, kind="ExternalInput")
    d_iidx = nc.dram_tensor("iidx", [128, T_TOT * 64], dt.int16, kind="ExternalInput")
    d_cidx = nc.dram_tensor("cidx", [128, T_TOT * 256], dt.int16, kind="ExternalInput")
    d_out = nc.dram_tensor("out", [128, T_TOT * 8], dt.float32, kind="ExternalOutput")

    with tile.TileContext(nc) as tc, ExitStack() as ctx:
        nc.gpsimd.load_library(library_config.mlp)
        consts = ctx.enter_context(tc.tile_pool(name="consts", bufs=1))
        dpool = ctx.enter_context(tc.tile_pool(name="dram", bufs=1, space="DRAM"))
        gpool = ctx.enter_context(tc.tile_pool(name="gath", bufs=2))
        spool = ctx.enter_context(tc.tile_pool(name="sig", bufs=2))
        pproj = ctx.enter_context(tc.tile_pool(name="pproj", bufs=2, space="PSUM"))
        pdot = ctx.enter_context(tc.tile_pool(name="pdot", bufs=2, space="PSUM"))

        # ---- constants into SBUF ----
        t_cT = consts.tile([EMB, N_CONC], dt.bfloat16)
        nc.gpsimd.dma_start(t_cT[:], d_cT.ap())
        t_wsa = consts.tile([EMB, 128], dt.bfloat16)
        nc.gpsimd.dma_start(t_wsa[:], d_wsa.ap())
        t_wsb = consts.tile([EMB, 128], dt.bfloat16)
        nc.gpsimd.dma_start(t_wsb[:], d_wsb.ap())
        t_wia = consts.tile([EMB, 128], dt.bfloat16)
        nc.gpsimd.dma_start(t_wia[:], d_wia.ap())
        t_wib = consts.tile([EMB, 128], dt.bfloat16)
        nc.gpsimd.dma_start(t_wib[:], d_wib.ap())
        t_w = consts.tile([128, 1], dt.bfloat16)
        nc.gpsimd.dma_start(t_w[:], d_w.ap())
        t_be = consts.tile([128, 1], dt.float32)
        nc.gpsimd.dma_start(t_be[:], d_be.ap())
        t_sidx = consts.tile([128, T_TOT * 64], dt.int16)
        nc.gpsimd.dma_start(t_sidx[:], d_sidx.ap())
        t_iidx = consts.tile([128, T_TOT * 64], dt.int16)
        nc.gpsimd.dma_start(t_iidx[:], d_iidx.ap())
        t_cidx = consts.tile([128, T_TOT * 256], dt.int16)
        nc.gpsimd.dma_start(t_cidx[:], d_cidx.ap())
        t_oacc = consts.tile([128, T_TOT * 8], dt.float32)

        # ---- setup: project concept tables into DRAM (bf16 rows [2048, 128]) ----
        d_cs = dpool.tile([N_CONC, 128], dt.bfloat16)
        d_ci = dpool.tile([N_CONC, 128], dt.bfloat16)
        for tbl, (wt, dst) in enumerate([(t_wsb, d_cs), (t_wib, d_ci)]):
            for g in range(NVG):  # 512 conc rows per group
                ps = pproj.tile([128, 512], dt.float32, tag="setup")
                for i in range(4):
                    v0 = 512 * g + 128 * i
                    nc.tensor.matmul(
                        ps[:, 128 * i : 128 * (i + 1)],
                        t_cT[:, v0 : v0 + 128],       # lhsT [64, 128v]
                        wt[:],                        # rhs  [64d, 128c]
                        start=True, stop=True,
                    )
                st = gpool.tile([128, 512], dt.bfloat16, tag="setupc")
                nc.vector.tensor_copy(st[:], ps[:])
                dst_ap = dst[512 * g : 512 * (g + 1), :].rearrange(
                    "(b p) c -> p b c", p=128)
                nc.gpsimd.dma_start(dst_ap, st[:].rearrange("p (b c) -> p b c", b=4))

        # ---- main loop over super-tiles ----
        for t in range(T_TOT):
            item_tbl = d_ilo if t < T_LO else d_ihi

            g_stu = gpool.tile([128, 1, TILE_E], dt.bfloat16)
            nc.gpsimd.dma_gather(g_stu[:], d_stu.ap(),
                                 t_sidx[:, 64 * t : 64 * (t + 1)],
                                 TILE_E, TILE_E, 128, elem_step=128, transpose=True,
                                 single_packet=False)
            g_item = gpool.tile([128, 1, TILE_E], dt.bfloat16)
            nc.gpsimd.dma_gather(g_item[:], item_tbl.ap(),
                                 t_iidx[:, 64 * t : 64 * (t + 1)],
                                 TILE_E, TILE_E, 128, elem_step=128, transpose=True,
                                 single_packet=False)
            g_cs = gpool.tile([128, 1, NPAIR], dt.bfloat16)
            nc.gpsimd.dma_gather(g_cs[:], d_cs[:],
                                 t_cidx[:, 256 * t : 256 * (t + 1)],
                                 NPAIR, NPAIR, 128, elem_step=128, transpose=True,
                                 single_packet=False)
            g_ci = gpool.tile([128, 1, NPAIR], dt.bfloat16)
            nc.gpsimd.dma_gather(g_ci[:], d_ci[:],
                                 t_cidx[:, 256 * t : 256 * (t + 1)],
                                 NPAIR, NPAIR, 128, elem_step=128, transpose=True,
                                 single_packet=False)

            # per-edge projections on PE: psum [128c, 1024e]
            ps_s = pproj.tile([128, TILE_E], dt.float32, tag="proj")
            for h in range(2):
                nc.tensor.matmul(ps_s[:, 512 * h : 512 * (h + 1)],
                                 t_wsa[:],
                                 g_stu[0:EMB, 0, 512 * h : 512 * (h + 1)],
                                 start=True, stop=True)
            ps_i = pproj.tile([128, TILE_E], dt.float32, tag="proj")
            for h in range(2):
                nc.tensor.matmul(ps_i[:, 512 * h : 512 * (h + 1)],
                                 t_wia[:],
                                 g_item[0:EMB, 0, 512 * h : 512 * (h + 1)],
                                 start=True, stop=True)
            part_s = gpool.tile([128, TILE_E], dt.bfloat16)
            nc.vector.tensor_copy(part_s[:], ps_s[:])
            part_i = gpool.tile([128, TILE_E], dt.bfloat16)
            nc.vector.tensor_copy(part_i[:], ps_i[:])

            # adds: sin = conc_s + stu_part (bcast over k), iin likewise
            t_sin = spool.tile([128, NPAIR], dt.bfloat16)
            nc.vector.tensor_add(
                t_sin[:].rearrange("p (k e) -> p k e", k=K),
                g_cs[:].rearrange("p one (k e) -> p (one k) e", k=K),
                part_s[:].unsqueeze(1).broadcast_to([128, K, TILE_E]))
            t_iin = spool.tile([128, NPAIR], dt.bfloat16)
            nc.vector.tensor_add(
                t_iin[:].rearrange("p (k e) -> p k e", k=K),
                g_ci[:].rearrange("p one (k e) -> p (one k) e", k=K),
                part_i[:].unsqueeze(1).broadcast_to([128, K, TILE_E]))

            # sigmoids on ACT (item side negated via scale=-1)
            t_ss = spool.tile([128, NPAIR], dt.bfloat16)
            nc.scalar.activation(t_ss[:], t_sin[:],
                                 mybir.ActivationFunctionType.Sigmoid, scale=1.0)
            t_si = spool.tile([128, NPAIR], dt.bfloat16)
            nc.scalar.activation(t_si[:], t_iin[:],
                                 mybir.ActivationFunctionType.Sigmoid, scale=-1.0)

            # dot with w: both sides accumulate into one PSUM column per block
            ps_d = pdot.tile([128, 32], dt.float32)
            for b in range(32):
                nc.tensor.matmul(ps_d[:, b : b + 1],
                                 t_ss[:, 128 * b : 128 * (b + 1)], t_w[:],
                                 start=True, stop=False)
                nc.tensor.matmul(ps_d[:, b : b + 1],
                                 t_si[:, 128 * b : 128 * (b + 1)], t_w[:],
                                 start=False, stop=True)

            # final sigmoid (bias = b - sum(w)) straight from PSUM
            t_pred = gpool.tile([128, 32], dt.bfloat16)
            nc.scalar.activation(t_pred[:], ps_d[:],
                                 mybir.ActivationFunctionType.Sigmoid,
                                 bias=t_be[:], scale=1.0)

            # k-mean: cols b = k*8 + j  ->  out_acc[:, 8t+j]
            t_m = gpool.tile([128, 8], dt.float32)
            nc.vector.reduce_sum(t_m[:],
                                 t_pred[:].rearrange("p (k j) -> p j k", j=8),
                                 axis=mybir.AxisListType.X)
            nc.vector.tensor_scalar_mul(t_oacc[:, 8 * t : 8 * (t + 1)], t_m[:], 0.25)

        nc.gpsimd.dma_start(d_out.ap(), t_oacc[:])

    nc.compile()
    return nc


def _prep_core(stu_i, item_i, conc_i):
    """Per-core host prep: partition by item range, pad, build wrapped idx."""
    lo_sel = np.nonzero(item_i < ITEM_SPLIT)[0]
    hi_sel = np.nonzero(item_i >= ITEM_SPLIT)[0]
    n_lo, n_hi = len(lo_sel), len(hi_sel)
    if n_lo > LO_CAP or n_hi > HI_CAP:
        return None  # fall back (statistically impossible for this distribution)

    stu16 = np.zeros(E_PAD, np.int16)
    item16 = np.zeros(E_PAD, np.int16)
    conc16 = np.zeros((E_PAD, K), np.int16)
    stu16[:n_lo] = stu_i[lo_sel]
    item16[:n_lo] = item_i[lo_sel]
    conc16[:n_lo] = conc_i[lo_sel]
    stu16[LO_CAP : LO_CAP + n_hi] = stu_i[hi_sel]
    item16[LO_CAP : LO_CAP + n_hi] = item_i[hi_sel] - ITEM_SPLIT
    conc16[LO_CAP : LO_CAP + n_hi] = conc_i[hi_sel]

    def wrap_e(a):  # [E_PAD] -> [128, T*64]
        w = a.reshape(T_TOT, 64, 16).transpose(0, 2, 1)      # [T, 16, 64]
        w = w.transpose(1, 0, 2).reshape(16, T_TOT * 64)
        return np.tile(w, (8, 1)).copy()

    cp = conc16.reshape(T_TOT, TILE_E, K).transpose(0, 2, 1)  # [T, K, 1024] k-major
    cp = cp.reshape(T_TOT, 256, 16).transpose(0, 2, 1)        # [T, 16, 256]
    cidx = np.tile(cp.transpose(1, 0, 2).reshape(16, T_TOT * 256), (8, 1)).copy()

    return (wrap_e(stu16), wrap_e(item16), cidx, lo_sel, hi_sel)


def _reference_np(stu_idx, item_idx, conc_idx, stu_fusion, item_fusion,
                  concept_fusion, W_stu, W_item, w_pred, b_pred):
    """Plain numpy fallback (only for astronomically unlikely cap overflow)."""
    emb = stu_fusion.shape[1]
    sp = (stu_fusion @ W_stu[:, :emb].T)[stu_idx]
    ip = (item_fusion @ W_item[:, :emb].T)[item_idx]
    cs = (concept_fusion @ W_stu[:, emb:].T)[conc_idx]
    ci = (concept_fusion @ W_item[:, emb:].T)[conc_idx]
    sig = lambda x: 1.0 / (1.0 + np.exp(-x))
    diff = sig(sp[:, None, :] + cs) - sig(ip[:, None, :] + ci)
    per = sig(diff @ w_pred + b_pred[0])
    return per.mean(axis=1).astype(np.float32)


def kernel(stu_idx, item_idx, conc_idx, stu_fusion, item_fusion,
           concept_fusion, W_stu, W_item, w_pred, b_pred):
    stu_idx = np.asarray(stu_idx, np.int64)
    item_idx = np.asarray(item_idx, np.int64)
    conc_idx = np.asarray(conc_idx, np.int64)
    stu_fusion = np.asarray(stu_fusion, np.float32)
    item_fusion = np.asarray(item_fusion, np.float32)
    concept_fusion = np.asarray(concept_fusion, np.float32)
    W_stu = np.asarray(W_stu, np.float32)
    W_item = np.asarray(W_item, np.float32)
    w_pred = np.asarray(w_pred, np.float32)
    b_pred = np.asarray(b_pred, np.float32)

    # ---- replicated table/weight prep (layout only) ----
    stu_pad = np.zeros((N_STU, 128), bf16)
    stu_pad[:, :EMB] = stu_fusion.astype(bf16)
    item_pad = np.zeros((N_ITEM, 128), bf16)
    item_pad[:, :EMB] = item_fusion.astype(bf16)
    item_lo = np.ascontiguousarray(item_pad[:ITEM_SPLIT])
    item_hi = np.ascontiguousarray(item_pad[ITEM_SPLIT:])
    concT = np.ascontiguousarray(concept_fusion.T).astype(bf16)      # [64, 2048]
    WT_s = np.ascontiguousarray(W_stu.T).astype(bf16)                # [128, 128]
    WT_i = np.ascontiguousarray(W_item.T).astype(bf16)
    WsaT, WsbT = np.ascontiguousarray(WT_s[:EMB]), np.ascontiguousarray(WT_s[EMB:])
    WiaT, WibT = np.ascontiguousarray(WT_i[:EMB]), np.ascontiguousarray(WT_i[EMB:])
    w_b = w_pred.astype(bf16).reshape(128, 1)
    beff = np.full((128, 1), b_pred[0] - w_pred.sum(), np.float32)

    in_maps = []
    perms = []
    for c in range(NCORES):
        sl = slice(c * E_CORE, (c + 1) * E_CORE)
        prep = _prep_core(stu_idx[sl], item_idx[sl], conc_idx[sl])
        if prep is None:
            return _reference_np(stu_idx, item_idx, conc_idx, stu_fusion,
                                 item_fusion, concept_fusion, W_stu, W_item,
                                 w_pred, b_pred)
        sidx, iidx, cidx, lo_sel, hi_sel = prep
        perms.append((lo_sel, hi_sel))
        in_maps.append({
            "stu_pad": stu_pad, "item_lo": item_lo, "item_hi": item_hi,
            "concT": concT, "WsaT": WsaT, "WsbT": WsbT, "WiaT": WiaT,
            "WibT": WibT, "w": w_b, "beff": beff,
            "sidx": sidx, "iidx": iidx, "cidx": cidx,
        })

    if "nc" not in _CACHE:
        _CACHE["nc"] = _build_nc()
    nc = _CACHE["nc"]

    res = run_bass_kernel_spmd(nc, in_maps, core_ids=list(range(NCORES)))

    out = np.empty(EDGES, np.float32)
    for c in range(NCORES):
        arr = np.asarray(res.results[c]["out"], np.float32)          # [128, 256]
        pad = arr.T.reshape(T_TOT, 8, 128).reshape(E_PAD)            # e_pad order
        lo_sel, hi_sel = perms[c]
        core_out = np.empty(E_CORE, np.float32)
        core_out[lo_sel] = pad[: len(lo_sel)]
        core_out[hi_sel] = pad[LO_CAP : LO_CAP + len(hi_sel)]
        out[c * E_CORE : (c + 1) * E_CORE] = core_out
    return out


# revision 5
# speedup vs baseline: 1.4709x; 1.4709x over previous
"""Trainium2 Bass kernel for nn_Diagnosis (gnn_message_passing).

Model (per edge e, concepts k=0..3):
  stu_part  = (stu_fusion  @ Ws_a.T)[stu_idx]           [E, 128]
  item_part = (item_fusion @ Wi_a.T)[item_idx]          [E, 128]
  conc_s    = (concept_fusion @ Ws_b.T)[conc_idx]       [E, 4, 128]
  conc_i    = (concept_fusion @ Wi_b.T)[conc_idx]       [E, 4, 128]
  pred[e,k] = sigmoid(w . (sig(stu_part+conc_s) - sig(item_part+conc_i)) + b)
  out[e]    = mean_k pred[e,k]

Device strategy (data-parallel over edges, 8 cores, all-bf16 on chip):
  - channels-on-partitions layout [128 c, pairs] via transpose-mode dma_gather
  - raw stu/item rows gathered from bf16-padded tables, projected per-tile on PE
  - concept tables projected once on device into DRAM, then row-gathered per pair
  - sig(stu)-sig(item) = sig(stu) + sig(-item) - 1: item side uses ACT scale=-1,
    the constant folds into the final bias  b_eff = b - sum(w)
  - dot with w via PE: lhsT = sigmoid tile [128c x 128pairs], rhs = w [128,1],
    both sides accumulated into one PSUM column -> final sigmoid reads PSUM
  - k-mean on DVE (strided reduce), output [128, 256] f32 per core
"""
import numpy as np
import ml_dtypes
from contextlib import ExitStack

import concourse.bacc as bacc
import concourse.tile as tile
import concourse.mybir as mybir
from concourse.bass_utils import run_bass_kernel_spmd
from concourse import library_config

bf16 = ml_dtypes.bfloat16

# ---- problem constants (hardcoded per contest rules) ----
N_STU, N_ITEM, N_CONC = 10000, 50000, 2048
EDGES, K, EMB, CNUM = 250000, 4, 64, 128
NCORES = 8
E_CORE = EDGES // NCORES          # 31250
ITEM_SPLIT = 32768                # int16 index range split for the item table
N_ITEM_HI = N_ITEM - ITEM_SPLIT   # 17232
TILE_E = 1024                     # edges per super-tile
T_LO, T_HI = 21, 11               # super-tiles for item<32768 / >=32768 groups
T_TOT = T_LO + T_HI               # 32
LO_CAP, HI_CAP = T_LO * TILE_E, T_HI * TILE_E   # 21504, 11264
E_PAD = T_TOT * TILE_E            # 32768 padded edges per core
NPAIR = 4 * TILE_E                # 4096 pairs per super-tile
NVG = N_CONC // 512               # conc projection groups of 512 rows

_CACHE = {}


def _build_nc():
    nc = bacc.Bacc("TRN2", target_bir_lowering=False, debug=False)
    dt = mybir.dt

    d_stu = nc.dram_tensor("stu_pad", [N_STU, 128], dt.bfloat16, kind="ExternalInput")
    d_ilo = nc.dram_tensor("item_lo", [ITEM_SPLIT, 128], dt.bfloat16, kind="ExternalInput")
    d_ihi = nc.dram_tensor("item_hi", [N_ITEM_HI, 128], dt.bfloat16, kind="ExternalInput")
    d_cT = nc.dram_tensor("concT", [EMB, N_CONC], dt.bfloat16, kind="ExternalInput")
    d_wsa = nc.dram_tensor("WsaT", [EMB, 128], dt.bfloat16, kind="ExternalInput")
    d_wsb = nc.dram_tensor("WsbT", [EMB, 128], dt.bfloat16, kind="ExternalInput")
    d_wia = nc.dram_tensor("WiaT", [EMB, 128], dt.bfloat16, kind="ExternalInput")
    d_wib = nc.dram_tensor("WibT", [EMB, 128], dt.bfloat16, kind="ExternalInput")
    d_w = nc.dram_tensor("w", [128, 1], dt.bfloat16, kind="ExternalInput")
    d_be = nc.dram_tensor("beff", [128, 1], dt.float32, kind="ExternalInput")
    d_sidx = nc.dram_tensor("sidx", [128, T_TOT * 64], dt.int16, kind="ExternalInput")
    d_iidx = nc.dram_tensor("iidx", [128, T_TOT * 64], dt.int16, kind="ExternalInput")
    d_cidx = nc.dram_tensor("cidx", [128, T_TOT * 256], dt.int16, kind="ExternalInput")
    d_out = nc.dram_tensor("out", [128, T_TOT * 8], dt.float32, kind="ExternalOutput")

    with tile.TileContext(nc) as tc, ExitStack() as ctx:
        nc.gpsimd.load_library(library_config.mlp)
        consts = ctx.enter_context(tc.tile_pool(name="consts", bufs=1))
        dpool = ctx.enter_context(tc.tile_pool(name="dram", bufs=1, space="DRAM"))
        gpool = ctx.enter_context(tc.tile_pool(name="gath", bufs=2))
        spool = ctx.enter_context(tc.tile_pool(name="sig", bufs=2))
        pproj = ctx.enter_context(tc.tile_pool(name="pproj", bufs=2, space="PSUM"))
        pdot = ctx.enter_context(tc.tile_pool(name="pdot", bufs=2, space="PSUM"))

        # ---- constants into SBUF ----
        t_cT = consts.tile([EMB, N_CONC], dt.bfloat16)
        nc.gpsimd.dma_start(t_cT[:], d_cT.ap())
        t_wsa = consts.tile([EMB, 128], dt.bfloat16)
        nc.gpsimd.dma_start(t_wsa[:], d_wsa.ap())
        t_wsb = consts.tile([EMB, 128], dt.bfloat16)
        nc.gpsimd.dma_start(t_wsb[:], d_wsb.ap())
        t_wia = consts.tile([EMB, 128], dt.bfloat16)
        nc.gpsimd.dma_start(t_wia[:], d_wia.ap())
        t_wib = consts.tile([EMB, 128], dt.bfloat16)
        nc.gpsimd.dma_start(t_wib[:], d_wib.ap())
        t_w = consts.tile([128, 1], dt.bfloat16)
        nc.gpsimd.dma_start(t_w[:], d_w.ap())
        t_be = consts.tile([128, 1], dt.float32)
        nc.gpsimd.dma_start(t_be[:], d_be.ap())
        t_sidx = consts.tile([128, T_TOT * 64], dt.int16)
        nc.gpsimd.dma_start(t_sidx[:], d_sidx.ap())
        t_iidx = consts.tile([128, T_TOT * 64], dt.int16)
        nc.gpsimd.dma_start(t_iidx[:], d_iidx.ap())
        t_cidx = consts.tile([128, T_TOT * 256], dt.int16)
        nc.gpsimd.dma_start(t_cidx[:], d_cidx.ap())
        t_oacc = consts.tile([128, T_TOT * 8], dt.float32)

        # ---- setup: project conc tables into ONE interleaved DRAM table ----
        # row v = [proj_s(v) 128ch | proj_i(v) 128ch] = 512B -> one gather/pair
        d_c = dpool.tile([N_CONC, 256], dt.bfloat16)
        for half, wt in enumerate([t_wsb, t_wib]):
            for g in range(NVG):  # 512 conc rows per group
                ps = pproj.tile([128, 512], dt.float32, tag="setup")
                for i in range(4):
                    v0 = 512 * g + 128 * i
                    nc.tensor.matmul(
                        ps[:, 128 * i : 128 * (i + 1)],
                        t_cT[:, v0 : v0 + 128],       # lhsT [64, 128v]
                        wt[:],                        # rhs  [64d, 128c]
                        start=True, stop=True,
                    )
                st = gpool.tile([128, 512], dt.bfloat16, tag="setupc")
                nc.vector.tensor_copy(st[:], ps[:])
                dst_ap = d_c[512 * g : 512 * (g + 1),
                             128 * half : 128 * (half + 1)].rearrange(
                    "(b p) c -> p b c", p=128)
                nc.gpsimd.dma_start(dst_ap, st[:].rearrange("p (b c) -> p b c", b=4))

        # ---- main loop over super-tiles ----
        for t in range(T_TOT):
            item_tbl = d_ilo if t < T_LO else d_ihi

            g_stu = gpool.tile([128, 1, TILE_E], dt.bfloat16)
            nc.gpsimd.dma_gather(g_stu[:], d_stu.ap(),
                                 t_sidx[:, 64 * t : 64 * (t + 1)],
                                 TILE_E, TILE_E, 128, elem_step=128, transpose=True,
                                 single_packet=False)
            g_item = gpool.tile([128, 1, TILE_E], dt.bfloat16)
            nc.gpsimd.dma_gather(g_item[:], item_tbl.ap(),
                                 t_iidx[:, 64 * t : 64 * (t + 1)],
                                 TILE_E, TILE_E, 128, elem_step=128, transpose=True,
                                 single_packet=False)
            g_c = gpool.tile([128, 2, NPAIR], dt.bfloat16)
            nc.gpsimd.dma_gather(g_c[:], d_c[:],
                                 t_cidx[:, 256 * t : 256 * (t + 1)],
                                 NPAIR, NPAIR, 256, elem_step=256, transpose=True,
                                 single_packet=False)

            # per-edge projections on PE: psum [128c, 1024e]
            ps_s = pproj.tile([128, TILE_E], dt.float32, tag="proj")
            for h in range(2):
                nc.tensor.matmul(ps_s[:, 512 * h : 512 * (h + 1)],
                                 t_wsa[:],
                                 g_stu[0:EMB, 0, 512 * h : 512 * (h + 1)],
                                 start=True, stop=True)
            ps_i = pproj.tile([128, TILE_E], dt.float32, tag="proj")
            for h in range(2):
                nc.tensor.matmul(ps_i[:, 512 * h : 512 * (h + 1)],
                                 t_wia[:],
                                 g_item[0:EMB, 0, 512 * h : 512 * (h + 1)],
                                 start=True, stop=True)
            part_s = gpool.tile([128, TILE_E], dt.bfloat16)
            nc.vector.tensor_copy(part_s[:], ps_s[:])
            part_i = gpool.tile([128, TILE_E], dt.bfloat16)
            nc.vector.tensor_copy(part_i[:], ps_i[:])

            # adds: sin = conc_s + stu_part (bcast over k), iin likewise
            t_sin = spool.tile([128, NPAIR], dt.bfloat16)
            nc.vector.tensor_add(
                t_sin[:].rearrange("p (k e) -> p k e", k=K),
                g_c[:, 0:1, :].rearrange("p one (k e) -> p (one k) e", k=K),
                part_s[:].unsqueeze(1).broadcast_to([128, K, TILE_E]))
            t_iin = spool.tile([128, NPAIR], dt.bfloat16)
            nc.vector.tensor_add(
                t_iin[:].rearrange("p (k e) -> p k e", k=K),
                g_c[:, 1:2, :].rearrange("p one (k e) -> p (one k) e", k=K),
                part_i[:].unsqueeze(1).broadcast_to([128, K, TILE_E]))

            # sigmoids on ACT (item side negated via scale=-1)
            t_ss = spool.tile([128, NPAIR], dt.bfloat16)
            nc.scalar.activation(t_ss[:], t_sin[:],
                                 mybir.ActivationFunctionType.Sigmoid, scale=1.0)
            t_si = spool.tile([128, NPAIR], dt.bfloat16)
            nc.scalar.activation(t_si[:], t_iin[:],
                                 mybir.ActivationFunctionType.Sigmoid, scale=-1.0)

            # dot with w: both sides accumulate into one PSUM column per block
            ps_d = pdot.tile([128, 32], dt.float32)
            for b in range(32):
                nc.tensor.matmul(ps_d[:, b : b + 1],
                                 t_ss[:, 128 * b : 128 * (b + 1)], t_w[:],
                                 start=True, stop=False)
                nc.tensor.matmul(ps_d[:, b : b + 1],
                                 t_si[:, 128 * b : 128 * (b + 1)], t_w[:],
                                 start=False, stop=True)

            # final sigmoid (bias = b - sum(w)) straight from PSUM
            t_pred = gpool.tile([128, 32], dt.bfloat16)
            nc.scalar.activation(t_pred[:], ps_d[:],
                                 mybir.ActivationFunctionType.Sigmoid,
                                 bias=t_be[:], scale=1.0)

            # k-mean: cols b = k*8 + j  ->  out_acc[:, 8t+j]
            t_m = gpool.tile([128, 8], dt.float32)
            nc.vector.reduce_sum(t_m[:],
                                 t_pred[:].rearrange("p (k j) -> p j k", j=8),
                                 axis=mybir.AxisListType.X)
            nc.vector.tensor_scalar_mul(t_oacc[:, 8 * t : 8 * (t + 1)], t_m[:], 0.25)

        nc.gpsimd.dma_start(d_out.ap(), t_oacc[:])

    nc.compile()
    return nc


def _prep_core(stu_i, item_i, conc_i):
    """Per-core host prep: partition by item range, pad, build wrapped idx."""
    lo_sel = np.nonzero(item_i < ITEM_SPLIT)[0]
    hi_sel = np.nonzero(item_i >= ITEM_SPLIT)[0]
    n_lo, n_hi = len(lo_sel), len(hi_sel)
    if n_lo > LO_CAP or n_hi > HI_CAP:
        return None  # fall back (statistically impossible for this distribution)

    stu16 = np.zeros(E_PAD, np.int16)
    item16 = np.zeros(E_PAD, np.int16)
    conc16 = np.zeros((E_PAD, K), np.int16)
    stu16[:n_lo] = stu_i[lo_sel]
    item16[:n_lo] = item_i[lo_sel]
    conc16[:n_lo] = conc_i[lo_sel]
    stu16[LO_CAP : LO_CAP + n_hi] = stu_i[hi_sel]
    item16[LO_CAP : LO_CAP + n_hi] = item_i[hi_sel] - ITEM_SPLIT
    conc16[LO_CAP : LO_CAP + n_hi] = conc_i[hi_sel]

    def wrap_e(a):  # [E_PAD] -> [128, T*64]
        w = a.reshape(T_TOT, 64, 16).transpose(0, 2, 1)      # [T, 16, 64]
        w = w.transpose(1, 0, 2).reshape(16, T_TOT * 64)
        return np.tile(w, (8, 1)).copy()

    cp = conc16.reshape(T_TOT, TILE_E, K).transpose(0, 2, 1)  # [T, K, 1024] k-major
    cp = cp.reshape(T_TOT, 256, 16).transpose(0, 2, 1)        # [T, 16, 256]
    cidx = np.tile(cp.transpose(1, 0, 2).reshape(16, T_TOT * 256), (8, 1)).copy()

    return (wrap_e(stu16), wrap_e(item16), cidx, lo_sel, hi_sel)


def _reference_np(stu_idx, item_idx, conc_idx, stu_fusion, item_fusion,
                  concept_fusion, W_stu, W_item, w_pred, b_pred):
    """Plain numpy fallback (only for astronomically unlikely cap overflow)."""
    emb = stu_fusion.shape[1]
    sp = (stu_fusion @ W_stu[:, :emb].T)[stu_idx]
    ip = (item_fusion @ W_item[:, :emb].T)[item_idx]
    cs = (concept_fusion @ W_stu[:, emb:].T)[conc_idx]
    ci = (concept_fusion @ W_item[:, emb:].T)[conc_idx]
    sig = lambda x: 1.0 / (1.0 + np.exp(-x))
    diff = sig(sp[:, None, :] + cs) - sig(ip[:, None, :] + ci)
    per = sig(diff @ w_pred + b_pred[0])
    return per.mean(axis=1).astype(np.float32)


def kernel(stu_idx, item_idx, conc_idx, stu_fusion, item_fusion,
           concept_fusion, W_stu, W_item, w_pred, b_pred):
    stu_idx = np.asarray(stu_idx, np.int64)
    item_idx = np.asarray(item_idx, np.int64)
    conc_idx = np.asarray(conc_idx, np.int64)
    stu_fusion = np.asarray(stu_fusion, np.float32)
    item_fusion = np.asarray(item_fusion, np.float32)
    concept_fusion = np.asarray(concept_fusion, np.float32)
    W_stu = np.asarray(W_stu, np.float32)
    W_item = np.asarray(W_item, np.float32)
    w_pred = np.asarray(w_pred, np.float32)
    b_pred = np.asarray(b_pred, np.float32)

    # ---- replicated table/weight prep (layout only) ----
    stu_pad = np.zeros((N_STU, 128), bf16)
    stu_pad[:, :EMB] = stu_fusion.astype(bf16)
    item_pad = np.zeros((N_ITEM, 128), bf16)
    item_pad[:, :EMB] = item_fusion.astype(bf16)
    item_lo = np.ascontiguousarray(item_pad[:ITEM_SPLIT])
    item_hi = np.ascontiguousarray(item_pad[ITEM_SPLIT:])
    concT = np.ascontiguousarray(concept_fusion.T).astype(bf16)      # [64, 2048]
    WT_s = np.ascontiguousarray(W_stu.T).astype(bf16)                # [128, 128]
    WT_i = np.ascontiguousarray(W_item.T).astype(bf16)
    WsaT, WsbT = np.ascontiguousarray(WT_s[:EMB]), np.ascontiguousarray(WT_s[EMB:])
    WiaT, WibT = np.ascontiguousarray(WT_i[:EMB]), np.ascontiguousarray(WT_i[EMB:])
    w_b = w_pred.astype(bf16).reshape(128, 1)
    beff = np.full((128, 1), b_pred[0] - w_pred.sum(), np.float32)

    in_maps = []
    perms = []
    for c in range(NCORES):
        sl = slice(c * E_CORE, (c + 1) * E_CORE)
        prep = _prep_core(stu_idx[sl], item_idx[sl], conc_idx[sl])
        if prep is None:
            return _reference_np(stu_idx, item_idx, conc_idx, stu_fusion,
                                 item_fusion, concept_fusion, W_stu, W_item,
                                 w_pred, b_pred)
        sidx, iidx, cidx, lo_sel, hi_sel = prep
        perms.append((lo_sel, hi_sel))
        in_maps.append({
            "stu_pad": stu_pad, "item_lo": item_lo, "item_hi": item_hi,
            "concT": concT, "WsaT": WsaT, "WsbT": WsbT, "WiaT": WiaT,
            "WibT": WibT, "w": w_b, "beff": beff,
            "sidx": sidx, "iidx": iidx, "cidx": cidx,
        })

    if "nc" not in _CACHE:
        _CACHE["nc"] = _build_nc()
    nc = _CACHE["nc"]

    res = run_bass_kernel_spmd(nc, in_maps, core_ids=list(range(NCORES)))

    out = np.empty(EDGES, np.float32)
    for c in range(NCORES):
        arr = np.asarray(res.results[c]["out"], np.float32)          # [128, 256]
        pad = arr.T.reshape(T_TOT, 8, 128).reshape(E_PAD)            # e_pad order
        lo_sel, hi_sel = perms[c]
        core_out = np.empty(E_CORE, np.float32)
        core_out[lo_sel] = pad[: len(lo_sel)]
        core_out[hi_sel] = pad[LO_CAP : LO_CAP + len(hi_sel)]
        out[c * E_CORE : (c + 1) * E_CORE] = core_out
    return out
